# revision 5
# baseline (speedup 1.0000x reference)
"""Trainium2 Bass kernel for nn_Decoder_76862734729499 (ragged masked MHA decoder).

Strategy
--------
Host side: fold the projection weights (the agent-id columns of Wk_proj add a
constant per (e,a) to every score, which cancels in softmax, so they are
dropped; the Wv agent-id column and the output projection fold into a single
per-agent bias).  All 512 (e,a) pairs are sorted by visited-length and dealt
snake-order across the 8 cores, so the per-slot lengths are nearly equal
across cores.  Each core receives its sequences packed back-to-back, zero
padded up to a 128-row tile boundary (zero rows score 0 -> exp 1, corrected by
subtracting the pad count from the softmax denominator).  One SPMD program
serves all cores.

Device side, per (e,a) slot:
  scores_T[l,h] = embT.T @ s_w      (PE; emb tile transposed on PE first)
  p = exp(scores_T)                 (ACT)
  ws[h,:]  = sum_l p[l,h]*emb[l,:]  (PE, PSUM accumulated over tiles)
  sumexp   = sum_l p[l,h]           (PE, ones column)
  out[e,a] = sum_h (ws[h]/Z_h) @ (Wvc_h @ Wo_h) + a * bias1
"""

import math
import numpy as np

E, A, L, D, H = 64, 8, 2048, 128, 8
DK = D // H
NCORES = 8
NPAIRS = (E * A) // NCORES  # 64 slots per core
TILE = 128
PACK = 4  # score/exp tiles packed per PSUM bank


def _host_prep(inputs):
    g = np.ascontiguousarray(np.asarray(inputs["graph_context"], np.float32))
    dep = np.ascontiguousarray(np.asarray(inputs["depot_embedding"], np.float32))
    tbd = np.ascontiguousarray(np.asarray(inputs["tbd_node_embedding"], np.float32))
    load = np.asarray(inputs["multi_current_load"], np.float32)
    emb = np.asarray(inputs["multi_visited_nodes_embeddings"], np.float32)
    lens = np.asarray(inputs["multi_visited_nodes_len"]).astype(np.int64)
    Wq_proj = np.asarray(inputs["Wq_proj"], np.float32)
    Wk_proj = np.asarray(inputs["Wk_proj"], np.float32)
    Wv_proj = np.asarray(inputs["Wv_proj"], np.float32)
    Wq = np.asarray(inputs["Wq"], np.float32)
    Wk = np.asarray(inputs["Wk"], np.float32)
    Wv = np.asarray(inputs["Wv"], np.float32)
    Wo = np.asarray(inputs["Wo"], np.float32)

    # ---- weight folds ----
    scale = 1.0 / math.sqrt(DK)
    Wkc = np.concatenate([Wk_proj[:D] @ Wk[h] for h in range(H)], axis=1) * scale
    Wvc = np.concatenate([Wv_proj[:D] @ Wv[h] for h in range(H)], axis=1)
    cv = np.concatenate([Wv_proj[D] @ Wv[h] for h in range(H)])
    Proj = np.stack([Wvc[:, h * DK:(h + 1) * DK] @ Wo[h] for h in range(H)])
    bias1 = sum(cv[h * DK:(h + 1) * DK] @ Wo[h] for h in range(H))
    Wq_stack = np.ascontiguousarray(np.transpose(Wq, (1, 0, 2)).reshape(D, D))
    # WkcT[h, k, d] = Wkc[d, h*DK + k]  (per-head stationary, K=DK partitions)
    WkcT = np.ascontiguousarray(
        Wkc.reshape(D, H, DK).transpose(1, 2, 0))  # [H, DK, D]

    # ---- pair assignment: sort by len desc, snake-deal across cores ----
    lens_flat = lens.reshape(-1)
    order = np.argsort(-lens_flat, kind="stable")
    perm = np.empty((NCORES, NPAIRS), np.int64)
    for s in range(NPAIRS):
        blk = order[s * NCORES:(s + 1) * NCORES]
        perm[:, s] = blk if s % 2 == 0 else blk[::-1]
    slot_len = lens_flat[perm]  # [NCORES, NPAIRS]
    n_tiles = np.maximum(1, np.ceil(slot_len.max(0) / TILE).astype(np.int64))
    tile_off = np.concatenate([[0], np.cumsum(n_tiles)])
    T_total = int(tile_off[-1])

    ctx = np.concatenate([g, dep, tbd], axis=1)  # [E, 3D]
    emb2 = emb.reshape(E * A, L, D)

    in_maps = []
    for c in range(NCORES):
        packed = np.zeros((T_total, TILE, D), np.float32)
        qin_T = np.empty((3 * D + 2, NPAIRS), np.float32)
        biasT = np.empty((D, NPAIRS), np.float32)
        npad = np.empty((1, NPAIRS), np.float32)
        for s in range(NPAIRS):
            pair = perm[c, s]
            e, a = divmod(int(pair), A)
            n = int(slot_len[c, s])
            flat = emb2[pair, :n]
            r0 = int(tile_off[s]) * TILE
            packed.reshape(T_total * TILE, D)[r0:r0 + n] = flat
            qin_T[:3 * D, s] = ctx[e]
            qin_T[3 * D, s] = load[e, a]
            qin_T[3 * D + 1, s] = float(a)
            biasT[:, s] = a * bias1
            npad[0, s] = -(int(n_tiles[s]) * TILE - n)
        in_maps.append({
            "embp": packed,
            "qin": qin_T,
            "biasv": biasT,
            "npad": np.broadcast_to(npad, (8, NPAIRS)).copy(),
            "wqp": Wq_proj,
            "wqs": Wq_stack,
            "wkc": WkcT,
            "proj": Proj,
            "ident": np.eye(D, dtype=np.float32),
        })
    return in_maps, perm, [int(x) for x in n_tiles], T_total


def build_program(n_tiles, T_total, npairs=NPAIRS):
    import concourse.bass as bass  # noqa: F401
    import concourse.tile as tile
    from concourse import bacc, mybir
    from contextlib import ExitStack

    f32 = mybir.dt.float32
    AF = mybir.ActivationFunctionType

    nc = bacc.Bacc("TRN2", target_bir_lowering=False, debug=False,
                   num_devices=NCORES)

    emb_d = nc.dram_tensor("embp", [T_total, TILE, D], f32, kind="ExternalInput")
    qin_d = nc.dram_tensor("qin", [3 * D + 2, npairs], f32, kind="ExternalInput")
    bias_d = nc.dram_tensor("biasv", [D, npairs], f32, kind="ExternalInput")
    npad_d = nc.dram_tensor("npad", [8, npairs], f32, kind="ExternalInput")
    wqp_d = nc.dram_tensor("wqp", [3 * D + 2, D], f32, kind="ExternalInput")
    wqs_d = nc.dram_tensor("wqs", [D, D], f32, kind="ExternalInput")
    wkc_d = nc.dram_tensor("wkc", [H, DK, D], f32, kind="ExternalInput")
    proj_d = nc.dram_tensor("proj", [H, D, D], f32, kind="ExternalInput")
    ident_d = nc.dram_tensor("ident", [D, D], f32, kind="ExternalInput")
    out_d = nc.dram_tensor("out", [npairs, D], f32, kind="ExternalOutput")

    qchunks = [(0, 128), (128, 128), (256, 128), (384, 2)]

    with ExitStack() as stk:
        tc = stk.enter_context(tile.TileContext(nc))
        const = stk.enter_context(tc.tile_pool(name="const", bufs=1))
        slab_p = stk.enter_context(tc.tile_pool(name="slab", bufs=2))
        tsb_p = stk.enter_context(tc.tile_pool(name="tsb", bufs=3))
        exp_p = stk.enter_context(tc.tile_pool(name="exp", bufs=3))
        small_p = stk.enter_context(tc.tile_pool(name="small", bufs=4))
        tps_p = stk.enter_context(
            tc.tile_pool(name="tps", bufs=2, space="PSUM"))
        scps_p = stk.enter_context(
            tc.tile_pool(name="scps", bufs=2, space="PSUM"))
        wsps_p = stk.enter_context(
            tc.tile_pool(name="wsps", bufs=2, space="PSUM"))
        mix_p = stk.enter_context(
            tc.tile_pool(name="mix", bufs=2, space="PSUM"))

        # ---- constants ----
        ident = const.tile([D, D], f32, tag="ident")
        nc.sync.dma_start(ident[:], ident_d.ap())
        wqs = const.tile([D, D], f32, tag="wqs")
        nc.sync.dma_start(wqs[:], wqs_d.ap())
        bias_sb = const.tile([D, npairs], f32, tag="bias")
        nc.sync.dma_start(bias_sb[:], bias_d.ap())
        npad_sb = const.tile([8, npairs], f32, tag="npad")
        nc.sync.dma_start(npad_sb[:], npad_d.ap())
        wkc_sb = []
        for h in range(H):
            t = const.tile([DK, D], f32, tag=f"wkc{h}")
            nc.sync.dma_start(t[:], wkc_d.ap()[h])
            wkc_sb.append(t)
        proj_sb = []
        for h in range(H):
            t = const.tile([D, D], f32, tag=f"proj{h}")
            nc.sync.dma_start(t[:], proj_d.ap()[h])
            proj_sb.append(t)
        qin_sb, wqp_sb = [], []
        for i, (o, k) in enumerate(qchunks):
            t = const.tile([k, npairs], f32, tag=f"qin{i}")
            nc.sync.dma_start(t[:], qin_d.ap()[o:o + k])
            qin_sb.append(t)
            t = const.tile([k, D], f32, tag=f"wqp{i}")
            nc.sync.dma_start(t[:], wqp_d.ap()[o:o + k])
            wqp_sb.append(t)
        s_w_all = const.tile([D, npairs, H], f32, tag="s_w_all")
        ws_allT = const.tile([D, npairs, H], f32, tag="ws_allT")

        # ---- prologue: q path ----
        q_ps = mix_p.tile([npairs, D], f32, tag="mix")
        for i in range(len(qchunks)):
            nc.tensor.matmul(q_ps[:], qin_sb[i][:], wqp_sb[i][:],
                             start=(i == 0), stop=(i == len(qchunks) - 1))
        q_sb = const.tile([npairs, D], f32, tag="q_sb")
        nc.vector.tensor_copy(q_sb[:], q_ps[:])

        qT_ps = mix_p.tile([D, npairs], f32, tag="mix")
        nc.tensor.transpose(qT_ps[:], q_sb[:], ident[0:npairs, 0:npairs])
        qT_sb = const.tile([D, npairs], f32, tag="qT_sb")
        nc.vector.tensor_copy(qT_sb[:], qT_ps[:])

        qh_ps = mix_p.tile([D, npairs], f32, tag="mix")
        nc.tensor.matmul(qh_ps[:], wqs[:], qT_sb[:], start=True, stop=True)
        qh_sb = const.tile([D, npairs], f32, tag="qh_sb")
        nc.vector.tensor_copy(qh_sb[:], qh_ps[:])

        # per-head slices of qh (partition-base relocation via SBUF->SBUF DMA)
        qh_h = []
        for h in range(H):
            t = const.tile([DK, npairs], f32, tag=f"qh{h}")
            nc.sync.dma_start(t[:], qh_sb[h * DK:(h + 1) * DK, :])
            qh_h.append(t)
        for h in range(H):
            sw_ps = mix_p.tile([D, npairs], f32, tag="mix")
            nc.tensor.matmul(sw_ps[:], wkc_sb[h][:], qh_h[h][:],
                             start=True, stop=True)
            nc.vector.tensor_copy(s_w_all[:, :, h], sw_ps[:])

        # ---- main loop over slots ----
        for s in range(npairs):
            nt = n_tiles[s]
            base = sum(n_tiles[:s])
            slab = slab_p.tile([TILE, nt, D + 1], f32, tag="slab")
            src = emb_d.ap()[base:base + nt].rearrange("t p d -> p t d")
            nc.sync.dma_start(slab[:, :, 0:D], src)
            nc.vector.memset(slab[:, :, D:D + 1], 1.0)

            ws_ps = wsps_p.tile([H, D + 1], f32, tag="wsps")
            npk = (nt + PACK - 1) // PACK
            for p in range(npk):
                t0 = p * PACK
                tn = min(PACK, nt - t0)
                tp_ps = tps_p.tile([TILE, PACK, D], f32, tag="tps")
                for j in range(tn):
                    nc.tensor.transpose(tp_ps[:, j, :],
                                        slab[:, t0 + j, 0:D], ident[:])
                tp_sb = tsb_p.tile([TILE, PACK, D], f32, tag="tsb")
                nc.vector.tensor_copy(tp_sb[:, 0:tn, :], tp_ps[:, 0:tn, :])
                sc_ps = scps_p.tile([TILE, PACK, H], f32, tag="scps")
                for j in range(tn):
                    nc.tensor.matmul(sc_ps[:, j, :], tp_sb[:, j, :],
                                     s_w_all[:, s, :], start=True, stop=True)
                ex_sb = exp_p.tile([TILE, PACK, H], f32, tag="exp")
                nc.scalar.activation(ex_sb[:, 0:tn, :], sc_ps[:, 0:tn, :],
                                     AF.Exp)
                for j in range(tn):
                    t = t0 + j
                    nc.tensor.matmul(ws_ps[:], ex_sb[:, j, :],
                                     slab[:, t, :],
                                     start=(t == 0), stop=(t == nt - 1))

            se = small_p.tile([H, 1], f32, tag="se")
            nc.vector.tensor_scalar_add(se[:], ws_ps[:, D:D + 1],
                                        npad_sb[:, s:s + 1])
            rec = small_p.tile([H, 1], f32, tag="rec")
            nc.vector.reciprocal(rec[:], se[:])
            ws_sb = small_p.tile([H, D], f32, tag="ws_sb")
            nc.vector.tensor_scalar_mul(ws_sb[:], ws_ps[:, 0:D], rec[:])
            wsT_ps = mix_p.tile([D, H], f32, tag="mix")
            nc.tensor.transpose(wsT_ps[:], ws_sb[:], ident[0:H, 0:H])
            nc.vector.tensor_copy(ws_allT[:, s, :], wsT_ps[:])

        # ---- final projection ----
        out_ps = mix_p.tile([D, npairs], f32, tag="mix")
        for h in range(H):
            nc.tensor.matmul(out_ps[:], proj_sb[h][:], ws_allT[:, :, h],
                             start=(h == 0), stop=(h == H - 1))
        out_sb = const.tile([D, npairs], f32, tag="out_sb")
        nc.vector.tensor_add(out_sb[:], out_ps[:], bias_sb[:])
        outT_ps = mix_p.tile([npairs, D], f32, tag="mix")
        nc.tensor.transpose(outT_ps[:], out_sb[:], ident[:])
        fin_sb = const.tile([npairs, D], f32, tag="fin_sb")
        nc.vector.tensor_copy(fin_sb[:], outT_ps[:])
        nc.sync.dma_start(out_d.ap(), fin_sb[:])

    nc.compile()
    return nc


def kernel(**inputs):
    from concourse.bass_utils import run_bass_kernel_spmd

    in_maps, perm, n_tiles, T_total = _host_prep(inputs)
    nc = build_program(n_tiles, T_total)
    res = run_bass_kernel_spmd(nc, in_maps, core_ids=list(range(NCORES)))

    full = np.empty((E * A, D), np.float32)
    for c in range(NCORES):
        full[perm[c]] = res.results[c]["out"]
    return full.reshape(E, A, D)


# revision 12
# speedup vs baseline: 1.4301x; 1.4301x over previous
"""Trainium2 Bass kernel for nn_Decoder_76862734729499 (ragged masked MHA decoder).

Strategy
--------
Host side: fold the projection weights (the agent-id columns of Wk_proj add a
constant per (e,a) to every score, which cancels in softmax, so they are
dropped; the Wv agent-id column and the output projection fold into a single
per-agent bias).  All 512 (e,a) pairs are sorted by visited-length and dealt
snake-order across the 8 cores, so the per-slot lengths are nearly equal
across cores.  Each core receives its sequences packed back-to-back, zero
padded up to a 128-row tile boundary (zero rows score 0 -> exp 1, corrected by
subtracting the pad count from the softmax denominator).  One SPMD program
serves all cores.

Device side, per (e,a) slot:
  scores_T[l,h] = embT.T @ s_w      (PE; emb tile transposed on PE first)
  p = exp(scores_T)                 (ACT)
  ws[h,:]  = sum_l p[l,h]*emb[l,:]  (PE, PSUM accumulated over tiles)
  sumexp   = sum_l p[l,h]           (PE, ones column)
  out[e,a] = sum_h (ws[h]/Z_h) @ (Wvc_h @ Wo_h) + a * bias1
"""

import math
import numpy as np

E, A, L, D, H = 64, 8, 2048, 128, 8
DK = D // H
NCORES = 8
NPAIRS = (E * A) // NCORES  # 64 slots per core
TILE = 128
PACK = 4  # score/exp tiles packed per PSUM bank
USE_BF16 = True  # bf16 tile path (cast during DMA); fp32 path if False


def _host_prep(inputs):
    g = np.ascontiguousarray(np.asarray(inputs["graph_context"], np.float32))
    dep = np.ascontiguousarray(np.asarray(inputs["depot_embedding"], np.float32))
    tbd = np.ascontiguousarray(np.asarray(inputs["tbd_node_embedding"], np.float32))
    load = np.asarray(inputs["multi_current_load"], np.float32)
    emb = np.asarray(inputs["multi_visited_nodes_embeddings"], np.float32)
    lens = np.asarray(inputs["multi_visited_nodes_len"]).astype(np.int64)
    Wq_proj = np.asarray(inputs["Wq_proj"], np.float32)
    Wk_proj = np.asarray(inputs["Wk_proj"], np.float32)
    Wv_proj = np.asarray(inputs["Wv_proj"], np.float32)
    Wq = np.asarray(inputs["Wq"], np.float32)
    Wk = np.asarray(inputs["Wk"], np.float32)
    Wv = np.asarray(inputs["Wv"], np.float32)
    Wo = np.asarray(inputs["Wo"], np.float32)

    # ---- weight folds ----
    scale = 1.0 / math.sqrt(DK)
    Wkc = np.concatenate([Wk_proj[:D] @ Wk[h] for h in range(H)], axis=1) * scale
    Wvc = np.concatenate([Wv_proj[:D] @ Wv[h] for h in range(H)], axis=1)
    cv = np.concatenate([Wv_proj[D] @ Wv[h] for h in range(H)])
    Proj = np.stack([Wvc[:, h * DK:(h + 1) * DK] @ Wo[h] for h in range(H)])
    bias1 = sum(cv[h * DK:(h + 1) * DK] @ Wo[h] for h in range(H))
    Wq_stack = np.ascontiguousarray(np.transpose(Wq, (1, 0, 2)).reshape(D, D))
    # WkcT[h, k, d] = Wkc[d, h*DK + k]  (per-head stationary, K=DK partitions)
    WkcT = np.ascontiguousarray(
        Wkc.reshape(D, H, DK).transpose(1, 2, 0))  # [H, DK, D]

    # ---- pair assignment: sort by len desc, snake-deal across cores ----
    lens_flat = lens.reshape(-1)
    order = np.argsort(-lens_flat, kind="stable")
    perm = np.empty((NCORES, NPAIRS), np.int64)
    for s in range(NPAIRS):
        blk = order[s * NCORES:(s + 1) * NCORES]
        perm[:, s] = blk if s % 2 == 0 else blk[::-1]
    slot_len = lens_flat[perm]  # [NCORES, NPAIRS]
    n_tiles = np.maximum(1, np.ceil(slot_len.max(0) / TILE).astype(np.int64))
    tile_off = np.concatenate([[0], np.cumsum(n_tiles)])
    T_total = int(tile_off[-1])

    ctx = np.concatenate([g, dep, tbd], axis=1)  # [E, 3D]
    emb2 = emb.reshape(E * A, L, D)

    in_maps = []
    for c in range(NCORES):
        packed = np.zeros((T_total, TILE, D), np.float32)
        qin_T = np.empty((3 * D + 2, NPAIRS), np.float32)
        biasT = np.empty((D, NPAIRS), np.float32)
        npad = np.empty((1, NPAIRS), np.float32)
        for s in range(NPAIRS):
            pair = perm[c, s]
            e, a = divmod(int(pair), A)
            n = int(slot_len[c, s])
            flat = emb2[pair, :n]
            r0 = int(tile_off[s]) * TILE
            packed.reshape(T_total * TILE, D)[r0:r0 + n] = flat
            qin_T[:3 * D, s] = ctx[e]
            qin_T[3 * D, s] = load[e, a]
            qin_T[3 * D + 1, s] = float(a)
            biasT[:, s] = a * bias1
            npad[0, s] = -(int(n_tiles[s]) * TILE - n)
        in_maps.append({
            "embp": packed,
            "qin": qin_T,
            "biasv": biasT,
            "npad": np.broadcast_to(npad, (8, NPAIRS)).copy(),
            "wqp": Wq_proj,
            "wqs": Wq_stack,
            "wkc": WkcT,
            "proj": Proj,
            "ident": np.eye(D, dtype=np.float32),
        })
    return in_maps, perm, [int(x) for x in n_tiles], T_total


def build_program(n_tiles, T_total, npairs=NPAIRS):
    import concourse.bass as bass  # noqa: F401
    import concourse.tile as tile
    from concourse import bacc, mybir
    from contextlib import ExitStack

    f32 = mybir.dt.float32
    bf16 = mybir.dt.bfloat16
    wdt = bf16 if USE_BF16 else f32
    AF = mybir.ActivationFunctionType

    nc = bacc.Bacc("TRN2", target_bir_lowering=False, debug=False,
                   num_devices=NCORES)

    emb_d = nc.dram_tensor("embp", [T_total, TILE, D], f32, kind="ExternalInput")
    qin_d = nc.dram_tensor("qin", [3 * D + 2, npairs], f32, kind="ExternalInput")
    bias_d = nc.dram_tensor("biasv", [D, npairs], f32, kind="ExternalInput")
    npad_d = nc.dram_tensor("npad", [8, npairs], f32, kind="ExternalInput")
    wqp_d = nc.dram_tensor("wqp", [3 * D + 2, D], f32, kind="ExternalInput")
    wqs_d = nc.dram_tensor("wqs", [D, D], f32, kind="ExternalInput")
    wkc_d = nc.dram_tensor("wkc", [H, DK, D], f32, kind="ExternalInput")
    proj_d = nc.dram_tensor("proj", [H, D, D], f32, kind="ExternalInput")
    ident_d = nc.dram_tensor("ident", [D, D], f32, kind="ExternalInput")
    out_d = nc.dram_tensor("out", [npairs, D], f32, kind="ExternalOutput")

    qchunks = [(0, 128), (128, 128), (256, 128), (384, 2)]

    with ExitStack() as stk:
        tc = stk.enter_context(tile.TileContext(nc))
        const = stk.enter_context(tc.tile_pool(name="const", bufs=1))
        slab_p = stk.enter_context(tc.tile_pool(name="slab", bufs=2))
        tsb_p = stk.enter_context(tc.tile_pool(name="tsb", bufs=3))
        exp_p = stk.enter_context(tc.tile_pool(name="exp", bufs=3))
        small_p = stk.enter_context(tc.tile_pool(name="small", bufs=4))
        tps_p = stk.enter_context(
            tc.tile_pool(name="tps", bufs=2, space="PSUM"))
        scps_p = stk.enter_context(
            tc.tile_pool(name="scps", bufs=2, space="PSUM"))
        wsps_p = stk.enter_context(
            tc.tile_pool(name="wsps", bufs=2, space="PSUM"))
        mix_p = stk.enter_context(
            tc.tile_pool(name="mix", bufs=2, space="PSUM"))

        # ---- constants ----
        ident = const.tile([D, D], f32, tag="ident")
        nc.sync.dma_start(ident[:], ident_d.ap())
        if USE_BF16:
            ident_w = const.tile([D, D], bf16, tag="identw")
            nc.vector.tensor_copy(ident_w[:], ident[:])
        else:
            ident_w = ident
        wqs = const.tile([D, D], f32, tag="wqs")
        nc.sync.dma_start(wqs[:], wqs_d.ap())
        bias_sb = const.tile([D, npairs], f32, tag="bias")
        nc.sync.dma_start(bias_sb[:], bias_d.ap())
        npad_sb = const.tile([8, npairs], f32, tag="npad")
        nc.sync.dma_start(npad_sb[:], npad_d.ap())
        wkc_sb = []
        for h in range(H):
            t = const.tile([DK, D], f32, tag=f"wkc{h}")
            nc.sync.dma_start(t[:], wkc_d.ap()[h])
            wkc_sb.append(t)
        proj_sb = []
        for h in range(H):
            t = const.tile([D, D], f32, tag=f"proj{h}")
            nc.sync.dma_start(t[:], proj_d.ap()[h])
            proj_sb.append(t)
        qin_sb, wqp_sb = [], []
        for i, (o, k) in enumerate(qchunks):
            t = const.tile([k, npairs], f32, tag=f"qin{i}")
            nc.sync.dma_start(t[:], qin_d.ap()[o:o + k])
            qin_sb.append(t)
            t = const.tile([k, D], f32, tag=f"wqp{i}")
            nc.sync.dma_start(t[:], wqp_d.ap()[o:o + k])
            wqp_sb.append(t)
        s_w_all = const.tile([D, npairs, H], f32, tag="s_w_all")
        ws_allT = const.tile([D, npairs, H], f32, tag="ws_allT")
        if USE_BF16:
            s_w_w = const.tile([D, npairs, H], bf16, tag="s_w_w")
        else:
            s_w_w = s_w_all

        # ---- prologue: q path ----
        q_ps = mix_p.tile([npairs, D], f32, tag="mix")
        for i in range(len(qchunks)):
            nc.tensor.matmul(q_ps[:], qin_sb[i][:], wqp_sb[i][:],
                             start=(i == 0), stop=(i == len(qchunks) - 1))
        q_sb = const.tile([npairs, D], f32, tag="q_sb")
        nc.vector.tensor_copy(q_sb[:], q_ps[:])

        qT_ps = mix_p.tile([D, npairs], f32, tag="mix")
        nc.tensor.transpose(qT_ps[:], q_sb[:], ident[0:npairs, 0:npairs])
        qT_sb = const.tile([D, npairs], f32, tag="qT_sb")
        nc.vector.tensor_copy(qT_sb[:], qT_ps[:])

        qh_ps = mix_p.tile([D, npairs], f32, tag="mix")
        nc.tensor.matmul(qh_ps[:], wqs[:], qT_sb[:], start=True, stop=True)
        qh_sb = const.tile([D, npairs], f32, tag="qh_sb")
        nc.vector.tensor_copy(qh_sb[:], qh_ps[:])

        # per-head slices of qh (partition-base relocation via SBUF->SBUF DMA)
        qh_h = []
        for h in range(H):
            t = const.tile([DK, npairs], f32, tag=f"qh{h}")
            nc.sync.dma_start(t[:], qh_sb[h * DK:(h + 1) * DK, :])
            qh_h.append(t)
        for h in range(H):
            sw_ps = mix_p.tile([D, npairs], f32, tag="mix")
            nc.tensor.matmul(sw_ps[:], wkc_sb[h][:], qh_h[h][:],
                             start=True, stop=True)
            nc.vector.tensor_copy(s_w_all[:, :, h], sw_ps[:])
        if USE_BF16:
            nc.vector.tensor_copy(s_w_w[:], s_w_all[:])

        # ---- main loop over slots ----
        for s in range(npairs):
            nt = n_tiles[s]
            base = sum(n_tiles[:s])
            slab = slab_p.tile([TILE, nt, D + 1], wdt, tag="slab")
            src = emb_d.ap()[base:base + nt].rearrange("t p d -> p t d")
            if USE_BF16:
                nc.gpsimd.dma_start(slab[:, :, 0:D], src)  # SWDGE casts f32->bf16
            else:
                nc.sync.dma_start(slab[:, :, 0:D], src)
            nc.vector.memset(slab[:, :, D:D + 1], 1.0)

            ws_ps = wsps_p.tile([H, D + 1], f32, tag="wsps")
            npk = (nt + PACK - 1) // PACK
            for p in range(npk):
                t0 = p * PACK
                tn = min(PACK, nt - t0)
                tp_ps = tps_p.tile([TILE, PACK, D], f32, tag="tps")
                for j in range(tn):
                    nc.tensor.matmul(tp_ps[:, j, :], slab[:, t0 + j, 0:D],
                                     ident_w[:], start=True, stop=True)
                tp_sb = tsb_p.tile([TILE, PACK, D], wdt, tag="tsb")
                nc.vector.tensor_copy(tp_sb[:, 0:tn, :], tp_ps[:, 0:tn, :])
                sc_ps = scps_p.tile([TILE, PACK, H], f32, tag="scps")
                for j in range(tn):
                    nc.tensor.matmul(sc_ps[:, j, :], tp_sb[:, j, :],
                                     s_w_w[:, s, :], start=True, stop=True)
                ex_sb = exp_p.tile([TILE, PACK, H], wdt, tag="exp")
                nc.scalar.activation(ex_sb[:, 0:tn, :], sc_ps[:, 0:tn, :],
                                     AF.Exp)
                for j in range(tn):
                    t = t0 + j
                    nc.tensor.matmul(ws_ps[:], ex_sb[:, j, :],
                                     slab[:, t, :],
                                     start=(t == 0), stop=(t == nt - 1))

            se = small_p.tile([H, 1], f32, tag="se")
            nc.vector.tensor_scalar_add(se[:], ws_ps[:, D:D + 1],
                                        npad_sb[:, s:s + 1])
            rec = small_p.tile([H, 1], f32, tag="rec")
            nc.vector.reciprocal(rec[:], se[:])
            ws_sb = small_p.tile([H, D], f32, tag="ws_sb")
            nc.vector.tensor_scalar_mul(ws_sb[:], ws_ps[:, 0:D], rec[:])
            wsT_ps = mix_p.tile([D, H], f32, tag="mix")
            nc.tensor.transpose(wsT_ps[:], ws_sb[:], ident[0:H, 0:H])
            nc.vector.tensor_copy(ws_allT[:, s, :], wsT_ps[:])

        # ---- final projection ----
        out_ps = mix_p.tile([D, npairs], f32, tag="mix")
        for h in range(H):
            nc.tensor.matmul(out_ps[:], proj_sb[h][:], ws_allT[:, :, h],
                             start=(h == 0), stop=(h == H - 1))
        out_sb = const.tile([D, npairs], f32, tag="out_sb")
        nc.vector.tensor_add(out_sb[:], out_ps[:], bias_sb[:])
        outT_ps = mix_p.tile([npairs, D], f32, tag="mix")
        nc.tensor.transpose(outT_ps[:], out_sb[:], ident[:])
        fin_sb = const.tile([npairs, D], f32, tag="fin_sb")
        nc.vector.tensor_copy(fin_sb[:], outT_ps[:])
        nc.sync.dma_start(out_d.ap(), fin_sb[:])

    nc.compile()
    return nc


def kernel(**inputs):
    from concourse.bass_utils import run_bass_kernel_spmd

    in_maps, perm, n_tiles, T_total = _host_prep(inputs)
    nc = build_program(n_tiles, T_total)
    res = run_bass_kernel_spmd(nc, in_maps, core_ids=list(range(NCORES)))

    full = np.empty((E * A, D), np.float32)
    for c in range(NCORES):
        full[perm[c]] = res.results[c]["out"]
    return full.reshape(E, A, D)
